# revision 5
# baseline (speedup 1.0000x reference)
"""Trainium2 Bass kernel v4 for the CRF scoring module (nn_CRF_14379550507279).

reference math:
    score0      = transitions[tags[:,0]] + emissions[:,0]            # (B,T)
    trans_steps = transitions[tags[:,:-1], tags[:,1:]] * mask[:,1:]  # (B,S-1)
    emit_steps  = emissions[:,1:,:] * mask[:,1:,None]                # (B,S-1,T)
    total = score0.sum() + trans_steps.sum()*T + emit_steps.sum()

Decomposition (per core, batch-parallel, 64 batches -> 128 partitions x 1024
(b,s) columns each):
  total = sum_{p,c} me[p,c] * R[p,c]      (R = sum_t emissions, bf16 tree)
        + 32 * <C, Tr>
  C[i,j] = sum_{p,c>=1} 1[prev_c=i] * 1[next_c=j] * mask_c  (+ specials)

v4 trans path: QUAD-BLOCK matmuls with a quad-native one-hot layout.
Columns c = kb*256 + cq; the one-hot of column c's tag is stored at
128-wide row index 32*kb + i:
    H[p, 32*kb+i, cq] = 1[prev_{cq+256kb} == i]     (prev shift baked in)
    Hb[p, 32*kb+j, cq] = 1[nm_{cq+256kb} == j]      (mask folded via nm)
One (128,128)x(128,128) bf16 matmul per cq contracts 4 columns' outer
products onto the four DIAGONAL 32x32 psum blocks (cross-column garbage
lands off-diagonal, never read). 255 quads + 5 stragglers replace 1027
per-column matmuls (PE SEQ was the bottleneck at ~42ns/column); build
cost is unchanged (64 tensor_scalar is_equal ops in packed 4x mode); v6
chunks the builds by cq-range and interleaves the quad matmuls so the PE
starts ~8us earlier instead of waiting for the full build; v7 alternates
quads between TWO psum accumulation chains to hide the psum
read-modify-write turnaround between back-to-back accumulating matmuls.

Engines: Pool does emissions tree pass1 (halves-add f32->bf16, frees the
DMA ring); DVE does one-hot builds + tree passes 2-5 + per-tile masked
accumulation; PE the quad matmuls; DMA streams emissions f32 (16.8MB/core,
the roofline). Host preps small bf16 tensors: nxt/nm/me, ohp0/oh0s, trt.
"""
import numpy as np

import concourse.bass as bass
import concourse.bacc as bacc
import concourse.mybir as mybir
import concourse.tile as tile
from concourse.bass_utils import run_bass_kernel_spmd

F32 = mybir.dt.float32
BF16 = mybir.dt.bfloat16
ALU = mybir.AluOpType
AXL = mybir.AxisListType

BF16NP = mybir.dt.np(BF16)

N_CORES = 8
B, S, T = 512, 2048, 32
BC = B // N_CORES          # 64 batches per core
P = 128                    # SBUF partitions
RPP = BC * S // P          # 1024 step-columns per partition
G = 128                    # emission (b,s) groups per tile
NT = RPP // G              # 8 emission tiles
QB = RPP // 4              # 256: quad block stride
NPOOL1 = 7                 # tiles whose tree-pass1 runs on Pool (rest DVE)

_cached = {}


def _build(repeat=1, do_emis=True, do_trans=True, npool1=NPOOL1,
           tree_depth=5, do_mm=True, dma_split=True, mm_contig=False,
           mm_stride=None):
    nc = bacc.Bacc("TRN2", target_bir_lowering=False, debug=False)

    ems = nc.dram_tensor("ems", [P, RPP, T], F32, kind="ExternalInput")
    nxt = nc.dram_tensor("nxt", [P, RPP], BF16, kind="ExternalInput")
    nmm = nc.dram_tensor("nmm", [P, RPP], BF16, kind="ExternalInput")
    mee = nc.dram_tensor("mee", [P, RPP], BF16, kind="ExternalInput")
    op0 = nc.dram_tensor("op0", [P, T], BF16, kind="ExternalInput")
    o0s = nc.dram_tensor("o0s", [P, T], BF16, kind="ExternalInput")
    io32 = nc.dram_tensor("io32", [P, T], BF16, kind="ExternalInput")
    trt = nc.dram_tensor("trt", [P, T], F32, kind="ExternalInput")
    out = nc.dram_tensor("out", [1, 1], F32, kind="ExternalOutput")

    with tile.TileContext(nc) as tc:
        with (
            tc.tile_pool(name="epool", bufs=3) as epool,
            tc.tile_pool(name="p1pool", bufs=3) as p1pool,
            tc.tile_pool(name="tpool", bufs=2) as tpool,
            tc.tile_pool(name="pers", bufs=1) as pers,
            tc.tile_pool(name="psum", bufs=1, space="PSUM") as psump,
        ):
          for _rep in range(repeat):
            # ---------- small loads (gpsimd SWDGE: keep HWDGE rings clean) ----
            nx = pers.tile([P, RPP], BF16, tag="nx")
            nc.gpsimd.dma_start(nx[:], nxt[:])
            nm = pers.tile([P, RPP], BF16, tag="nm")
            nc.gpsimd.dma_start(nm[:], nmm[:])
            me = pers.tile([P, RPP], BF16, tag="me")
            nc.gpsimd.dma_start(me[:], mee[:])
            ohp0 = pers.tile([P, T], BF16, tag="ohp0")
            nc.gpsimd.dma_start(ohp0[:], op0[:])
            oh0s = pers.tile([P, T], BF16, tag="oh0s")
            nc.gpsimd.dma_start(oh0s[:], o0s[:])
            iot = pers.tile([P, T], BF16, tag="iot")
            nc.gpsimd.dma_start(iot[:], io32[:])
            trtt = pers.tile([P, T], F32, tag="trtt")
            nc.gpsimd.dma_start(trtt[:], trt[:])
            ones32 = pers.tile([P, T], BF16, tag="ones32")
            nc.gpsimd.memset(ones32[:], 1.0)

            # ---------- emissions DMA (the big stream, 2 HWDGE queues) -------
            ets = []
            for j in range(NT if do_emis else 0):
                et = epool.tile([P, G * T], F32, tag="et")
                eng = nc.scalar if (dma_split and j % 2) else nc.sync
                eng.dma_start(
                    et[:].rearrange("p (g t) -> p g t", t=T),
                    ems[:, j * G:(j + 1) * G, :])
                ets.append(et)

            # ---------- one-hot builds (DVE, bf16 packed 4x) ----------
            # H[p, 32kb+i, cq]  = (nx[p, cq-1+256kb] == i)   for cq in [1,256)
            # Hb[p, 32kb+j, cq] = (nm[p, cq+256kb]   == j)   for cq in [1,256)
            H = pers.tile([P, P * QB], BF16, tag="H")
            Hb = pers.tile([P, P * QB], BF16, tag="Hb")
            # (kb, i, cq) views for the builds
            H4 = H[:].rearrange("p (kb i cq) -> p i kb cq", kb=4, i=T)
            Hb4 = Hb[:].rearrange("p (kb i cq) -> p i kb cq", kb=4, i=T)
            nx4 = nx[:].rearrange("p (kb cq) -> p kb cq", kb=4)
            nm4 = nm[:].rearrange("p (kb cq) -> p kb cq", kb=4)
            # (i128, cq) views for the matmuls
            Hv = H[:].rearrange("p (i128 cq) -> p i128 cq", cq=QB)
            Hbv = Hb[:].rearrange("p (i128 cq) -> p i128 cq", cq=QB)
            HCH = QB // 2            # build/matmul chunk boundary
            if do_trans:
                for lo, hi in ((1, HCH), (HCH, QB)):
                    for i in range(T):
                        nc.vector.tensor_scalar(
                            H4[:, i, :, lo:hi], nx4[:, :, lo - 1:hi - 1],
                            float(i), None, ALU.is_equal)
                        nc.vector.tensor_scalar(
                            Hb4[:, i, :, lo:hi], nm4[:, :, lo:hi],
                            float(i), None, ALU.is_equal)
                # straggler one-hots (broadcast is_equal vs iota, tiny):
                # prev one-hots for c in {256,512,768}; next/masked for same
                stg = pers.tile([P, 6 * T], BF16, tag="stg")
                stgv = stg[:].rearrange("p (s t) -> p s t", t=T)
                for k, c in enumerate((QB, 2 * QB, 3 * QB)):
                    nc.vector.tensor_tensor(
                        stgv[:, 2 * k:2 * k + 1],
                        nx[:, c - 1:c].broadcast_to((P, 1, T)),
                        iot[:].rearrange("p (s t) -> p s t", s=1),
                        ALU.is_equal)
                    nc.vector.tensor_tensor(
                        stgv[:, 2 * k + 1:2 * k + 2],
                        nm[:, c:c + 1].broadcast_to((P, 1, T)),
                        iot[:].rearrange("p (s t) -> p s t", s=1),
                        ALU.is_equal)
                # next0 one-hot for the c=0 step (rhs of ohp0 matmul)
                oh_n0 = pers.tile([P, T], BF16, tag="oh_n0")
                nc.vector.tensor_tensor(
                    oh_n0[:].rearrange("p (s t) -> p s t", s=1),
                    nx[:, 0:1].broadcast_to((P, 1, T)),
                    iot[:].rearrange("p (s t) -> p s t", s=1),
                    ALU.is_equal)

            # ---------- emissions tree (Pool pass1 + DVE passes 2-5) ----------
            R = pers.tile([P, RPP], BF16, tag="R")
            eaccs = pers.tile([P, NT], F32, tag="eaccs")
            if tree_depth < 5:
                nc.vector.memset(R[:], 0.0)
            if tree_depth < 5 or not do_emis:
                nc.vector.memset(eaccs[:], 0.0)
            for j, et in enumerate(ets):
                etv = et[:].rearrange("p (g t) -> p g t", t=T)
                if tree_depth < 1:
                    continue
                p1 = p1pool.tile([P, G * 16], BF16, tag="p1")
                p1v = p1[:].rearrange("p (g t) -> p g t", t=16)
                eng = nc.gpsimd if j < npool1 else nc.vector
                eng.tensor_tensor(p1v, etv[:, :, 0:16], etv[:, :, 16:32],
                                  ALU.add)
                if tree_depth < 2:
                    continue
                p2 = tpool.tile([P, G * 8], BF16, tag="p2")
                p2v = p2[:].rearrange("p (g t) -> p g t", t=8)
                nc.vector.tensor_tensor(p2v, p1v[:, :, 0:8], p1v[:, :, 8:16],
                                        ALU.add)
                if tree_depth < 3:
                    continue
                p3 = tpool.tile([P, G * 4], BF16, tag="p3")
                p3v = p3[:].rearrange("p (g t) -> p g t", t=4)
                nc.vector.tensor_tensor(p3v, p2v[:, :, 0:4], p2v[:, :, 4:8],
                                        ALU.add)
                if tree_depth < 4:
                    continue
                p4 = tpool.tile([P, G * 2], BF16, tag="p4")
                p4v = p4[:].rearrange("p (g t) -> p g t", t=2)
                nc.vector.tensor_tensor(p4v, p3v[:, :, 0:2], p3v[:, :, 2:4],
                                        ALU.add)
                if tree_depth < 5:
                    continue
                nc.vector.tensor_tensor(R[:, j * G:(j + 1) * G],
                                        p4v[:, :, 0], p4v[:, :, 1], ALU.add)
                escr = tpool.tile([P, G], BF16, tag="escr")
                nc.vector.tensor_tensor(escr[:], R[:, j * G:(j + 1) * G],
                                        me[:, j * G:(j + 1) * G], ALU.mult)
                nc.vector.tensor_reduce(eaccs[:, j:j + 1], escr[:],
                                        axis=AXL.X, op=ALU.add)

            # ---------- histogram matmuls (PE, bf16, quad blocks) ----------
            psC = psump.tile([P, P], F32, tag="psC")     # quad out (128,128)
            psC2 = psump.tile([P, P], F32, tag="psC2")   # second quad chain
            psD = psump.tile([P, T], F32, tag="psD")     # stragglers
            if do_trans and do_mm:
                for q in range(1, QB):
                    tgt = psC if q % 2 else psC2
                    nc.tensor.matmul(
                        tgt[:], Hv[:, :, q], Hbv[:, :, q],
                        start=(q in (1, 2)), stop=(q in (QB - 2, QB - 1)),
                        skip_group_check=True,
                        tile_position=(0, 0))
                # stragglers into psD 32-col strips (tile_position groups):
                # g0: c=0 (cross-partition prev, mask in ohp0) then c=768
                nc.tensor.matmul(psD[0:T, :], ohp0[:], oh_n0[:],
                                 start=True, stop=False, tile_position=(0, 0))
                nc.tensor.matmul(psD[0:T, :], stgv[:, 4], stgv[:, 5],
                                 start=False, stop=True, tile_position=(0, 0))
                # g1: score0 row-lookups (oh0s is pre-divided by 32)
                nc.tensor.matmul(psD[T:2 * T, :], oh0s[:], ones32[:],
                                 start=True, stop=True, tile_position=(0, T))
                # g2: c=256; g3: c=512
                nc.tensor.matmul(psD[2 * T:3 * T, :], stgv[:, 0], stgv[:, 1],
                                 start=True, stop=True, tile_position=(0, 2 * T))
                nc.tensor.matmul(psD[3 * T:4 * T, :], stgv[:, 2], stgv[:, 3],
                                 start=True, stop=True, tile_position=(0, 3 * T))
            else:
                nc.vector.memset(psC[:], 0.0)
                nc.vector.memset(psD[:], 0.0)
                if do_trans:
                    # keep builds live so they aren't dead-code'd
                    edump0 = pers.tile([P, T], BF16, tag="edump0")
                    nc.vector.tensor_copy(edump0[:], Hv[:, 0:T, 5])
                    nc.vector.tensor_copy(edump0[:], Hbv[:, 0:T, 5])

            # ---------- final combine ----------
            eacc = pers.tile([P, 1], F32, tag="eacc")
            nc.vector.tensor_reduce(eacc[:], eaccs[:], axis=AXL.X, op=ALU.add)
            # diag blocks of psC+psC2 (+ psD strip), dotted with Tr
            # (an op may read at most ONE psum input: copy psC2 out first)
            csb = pers.tile([P, T], F32, tag="csb")
            c2s = pers.tile([P, T], F32, tag="c2s")
            dsum = pers.tile([P, T], F32, tag="dsum")
            for k in range(4):
                nc.vector.tensor_copy(
                    c2s[T * k:T * (k + 1), :],
                    psC2[T * k:T * (k + 1), T * k:T * (k + 1)])
            for k in range(4):
                nc.vector.tensor_tensor(
                    dsum[T * k:T * (k + 1), :],
                    psC[T * k:T * (k + 1), T * k:T * (k + 1)],
                    c2s[T * k:T * (k + 1), :], ALU.add)
            nc.vector.tensor_tensor(csb[:], dsum[:], trtt[:], ALU.mult)
            csd = pers.tile([P, T], F32, tag="csd")
            nc.vector.tensor_tensor(csd[:], psD[:], trtt[:], ALU.mult)
            nc.vector.tensor_tensor(csb[:], csb[:], csd[:], ALU.add)
            ctr = pers.tile([P, 1], F32, tag="ctr")
            nc.vector.tensor_reduce(ctr[:], csb[:], axis=AXL.X, op=ALU.add)
            fin = pers.tile([P, 1], F32, tag="fin")
            nc.vector.scalar_tensor_tensor(
                out=fin[:], in0=ctr[:], scalar=32.0, in1=eacc[:],
                op0=ALU.mult, op1=ALU.add)
            ones = pers.tile([P, 1], F32, tag="ones")
            nc.gpsimd.memset(ones[:], 1.0)
            ps = psump.tile([1, 1], F32, tag="ps")
            nc.tensor.matmul(ps[:], ones[:], fin[:], start=True, stop=True)
            osb = pers.tile([1, 1], F32, tag="osb")
            nc.vector.tensor_copy(osb[:], ps[:])
            nc.sync.dma_start(out[:], osb[:])
    nc.compile()
    return nc


def _in_maps(emissions, tags, mask, transitions):
    trt = np.ascontiguousarray(np.tile(transitions, (4, 1)), np.float32)
    parity = np.arange(P) % 2                      # 0: s in [0,1024), 1: rest
    even = parity == 0
    eye = np.eye(T, dtype=np.float32)
    io32 = np.broadcast_to(np.arange(T, dtype=np.float32), (P, T))
    maps = []
    for c in range(N_CORES):
        sl = slice(c * BC, (c + 1) * BC)
        tg = np.ascontiguousarray(tags[sl]).reshape(P, RPP)
        mk = np.ascontiguousarray(mask[sl]).reshape(P, RPP).astype(np.float32)
        # transitions-step mask: col 0 invalid on even partitions (s=0)
        mt = mk.copy()
        mt[even, 0] = 0.0
        nm = (tg + 1.0) * mt - 1.0
        nm[:, 0] = -1.0                 # B col 0 never consumed by c>=1 path
        # emissions weight: s=0 (even partitions col 0) always counted
        me = mk.copy()
        me[even, 0] = 1.0
        # cross-partition c=0 prev one-hot (odd partitions), mask folded
        ohp0 = np.zeros((P, T), np.float32)
        prev0 = tg[:-1, RPP - 1]        # prev tag for partition p+1
        ohp0[1:] = eye[prev0] * mk[1:, 0:1]
        ohp0[even] = 0.0
        # score0 transition-row lookups, pre-divided by 32 (histogram is x32)
        oh0s = np.zeros((P, T), np.float32)
        oh0s[even] = eye[tg[even, 0]] / 32.0
        maps.append(dict(
            ems=np.ascontiguousarray(emissions[sl]).reshape(P, RPP, T),
            nxt=tg.astype(BF16NP),
            nmm=nm.astype(BF16NP),
            mee=me.astype(BF16NP),
            op0=ohp0.astype(BF16NP),
            o0s=oh0s.astype(BF16NP),
            io32=io32.astype(BF16NP),
            trt=trt,
        ))
    return maps


def kernel(emissions, tags, mask, transitions):
    emissions = np.asarray(emissions, np.float32)
    tags = np.asarray(tags, np.int32)
    mask = np.asarray(mask, np.float32)
    transitions = np.asarray(transitions, np.float32)

    if "nc" not in _cached:
        _cached["nc"] = _build()
    nc = _cached["nc"]
    maps = _in_maps(emissions, tags, mask, transitions)
    res = run_bass_kernel_spmd(nc, maps, list(range(N_CORES)))
    total = np.float64(0.0)
    for c in range(N_CORES):
        total += np.float64(res.results[c]["out"][0, 0])
    return np.float32(total)
